# revision 1
# baseline (speedup 1.0000x reference)
import numpy as np

# WPE dereverberation (nn_DNN_WPE_85177791414850)
# Shapes hardcoded per spec: B=8, F=257, C=8, T=800
TAPS = 5
DELAY = 3
EPS_POWER = 1e-7
EPS_REG = 1e-10


def _ytilde(Y, taps, delay):
    # Y: (BF, C, T) complex -> (BF, taps*C, T)
    BF, C, T = Y.shape
    Yp = np.pad(Y, ((0, 0), (0, 0), (delay + taps - 1, 0)))
    tilde = np.stack(
        [Yp[..., taps - 1 - k: taps - 1 - k + T] for k in range(taps)], axis=1
    )  # (BF, taps, C, T)
    return tilde.reshape(BF, taps * C, T)


def kernel(data_sep_real, data_sep_imag, data_mix_real, data_mix_imag, ilens):
    B, F, C, T = data_mix_real.shape
    BF = B * F

    sep_re = np.asarray(data_sep_real, np.float32).reshape(BF, C, T)
    sep_im = np.asarray(data_sep_imag, np.float32).reshape(BF, C, T)
    power = np.mean(sep_re * sep_re + sep_im * sep_im, axis=1)  # (BF, T)
    inv_power = (1.0 / np.maximum(power, EPS_POWER)).astype(np.float32)

    Y = (np.asarray(data_mix_real, np.float32) +
         1j * np.asarray(data_mix_imag, np.float32)).astype(np.complex64)
    Y = Y.reshape(BF, C, T)

    t0 = DELAY + TAPS - 1
    Yt = _ytilde(Y, TAPS, DELAY)                     # (BF, K, T)
    K = TAPS * C
    Ytv = Yt[..., t0:]                               # (BF, K, T')
    Yv = Y[..., t0:]                                 # (BF, C, T')
    w = inv_power[:, None, t0:]                      # (BF, 1, T')
    Ytw = (Ytv * w).astype(np.complex64)

    # R = Ytw @ Ytv^H  (BF, K, K);  P = Ytw @ Yv^H  (BF, K, C)
    YtvH = np.conj(np.transpose(Ytv, (0, 2, 1)))
    YvH = np.conj(np.transpose(Yv, (0, 2, 1)))
    R = np.matmul(Ytw, YtvH)
    P = np.matmul(Ytw, YvH)
    R = R + (EPS_REG * np.eye(K, dtype=np.complex64))[None]

    G = np.linalg.solve(R.astype(np.complex128), P.astype(np.complex128))
    G = G.astype(np.complex64)                       # (BF, K, C)

    # X = Y - G^H @ Yt
    GH = np.conj(np.transpose(G, (0, 2, 1)))         # (BF, C, K)
    X = Y - np.matmul(GH, Yt)                        # (BF, C, T)
    X = X.reshape(B, F, C, T)

    t = np.arange(T)
    valid = t[None, :] < np.asarray(ilens, np.int64)[:, None]     # (B, T)
    X = np.where(valid[:, None, None, :], X, 0)

    out = np.stack([X.real.astype(np.float32), X.imag.astype(np.float32)], axis=-1)
    return out

